# revision 1
# baseline (speedup 1.0000x reference)
"""Trainium2 Bass kernel for a 2-layer GCN (DGL GraphConv, norm='both').

Reference computation (per layer):
    h = relu( deg_in^-0.5 * segment_sum( ((x * deg_out^-0.5) @ W)[src], dst ) + b )
then logits = h2 @ Wc + bc.

Distribution: nodes are relabeled into 128-wide blocks, blocks are
load-balanced across the 8 NeuronCores (snake assignment by edge count),
giving every core an equal, structurally identical workload (SPMD: one
program, per-core data). Per layer:
  stage A: each core computes g = (x @ W) * s_out for its node shard
  AllGather: g shards -> full g table in every core's DRAM
  stage B: blocks are processed in groups of 4; per group, edge messages
    are gathered row-wise from the g table with one dma_gather per int16
    sub-table chunk; the per-block segment-sum is one-hot x messages
    matmuls accumulated in PSUM; epilogue scales by s_in, transposes,
    adds bias, relu -> h^T kept in SBUF.
Layer 2's epilogue is fused with the classifier: logits = h2 @ Wc + bc,
written per shard; the host reassembles and inverse-permutes.

All index preprocessing (degree counts, edge sorting/padding, relabeling)
is host-side numpy on integer graph structure; float math is on device.
"""
import math
from dataclasses import dataclass

import numpy as np

import concourse.bacc as bacc
import concourse.mybir as mybir
import concourse.tile as tile
from concourse.bass_utils import run_bass_kernel_spmd

f32 = mybir.dt.float32
bf16 = mybir.dt.bfloat16
i16 = mybir.dt.int16

P = 128  # partitions / node block size

# numpy view of bfloat16 for host-side constant/input arrays
import ml_dtypes  # noqa: E402  (ships with jax)

np_bf16 = ml_dtypes.bfloat16


@dataclass
class Cfg:
    n_nodes: int = 100000
    in_feats: int = 128
    num_classes: int = 4
    n_cores: int = 8
    nb: int = 98          # node blocks per core
    chunk: int = 25088    # gather sub-table rows (int16-addressable)
    group: int = 2        # blocks per gather group

    @property
    def npc(self):        # nodes per core
        return self.nb * P

    @property
    def npad(self):       # padded node count
        return self.n_cores * self.npc

    @property
    def n_chunks(self):
        return math.ceil(self.npad / self.chunk)

    @property
    def n_groups(self):
        return math.ceil(self.nb / self.group)


CFG = Cfg()


class Geometry:
    """Static slot layout derived from per-(block position, chunk) subtile
    capacities. Identical across cores (capacities are cross-core maxima)."""

    def __init__(self, cfg: Cfg, cap: np.ndarray):
        self.cap = cap  # [nb, n_chunks]
        G, NG, NCH = cfg.group, cfg.n_groups, cfg.n_chunks
        self.grp_blocks = [list(range(g * G, min((g + 1) * G, cfg.nb)))
                           for g in range(NG)]
        # per group: subtile base of (chunk, block-in-group), chunk ranges
        self.sub_base = []   # [NG][NCH][len(grp)] -> subtile index
        self.chunk_rng = []  # [NG][NCH] -> (s0, s1)
        self.Rg = []         # [NG] total subtiles
        for g in range(NG):
            blocks = self.grp_blocks[g]
            base = []
            rng = []
            s = 0
            for c in range(NCH):
                row = []
                c0 = s
                for b in blocks:
                    row.append(s)
                    s += int(cap[b, c])
                base.append(row)
                rng.append((c0, s))
            self.sub_base.append(base)
            self.chunk_rng.append(rng)
            self.Rg.append(s)
        self.Rmax = max(self.Rg)
        self.capmax = int(cap.max())
        # per block: ordered list of (group-subtile start, count) segments
        self.blk_segs = [[] for _ in range(cfg.nb)]
        for g in range(NG):
            for c in range(NCH):
                for i, b in enumerate(self.grp_blocks[g]):
                    n = int(cap[b, c])
                    if n:
                        self.blk_segs[b].append((self.sub_base[g][c][i], n))


def preprocess(cfg: Cfg, src: np.ndarray, dst: np.ndarray):
    """Relabel nodes, sort/pad edges into grouped gather metadata.

    Returns (geom, node_new, idx16, loc):
      idx16[m]: [n_groups, P, Rmax*8] int16 indices (16-wrapped, 8x repl)
      loc[m]:   [n_groups, P, Rmax]  f32 local dst in 0..127, 1000 for pads
    Group slot j -> partition j%128, group subtile j//128.
    """
    ncores, nb, nch, CH = cfg.n_cores, cfg.nb, cfg.n_chunks, cfg.chunk
    G = cfg.group
    n_blocks = ncores * nb

    # block load balancing: snake-assign blocks by edge count
    blk_tot = np.bincount(dst >> 7, minlength=n_blocks)
    order = np.argsort(-blk_tot, kind="stable")
    rank = np.arange(n_blocks)
    lane = rank % ncores
    rev = (rank // ncores) % 2 == 1
    core_of_rank = np.where(rev, ncores - 1 - lane, lane)
    core_of_old = np.empty(n_blocks, np.int64)
    pos_of_old = np.empty(n_blocks, np.int64)
    core_of_old[order] = core_of_rank
    pos_of_old[order] = rank // ncores
    new_blk_of_old = core_of_old * nb + pos_of_old
    node_ar = np.arange(cfg.npad, dtype=np.int64)
    node_new = new_blk_of_old[node_ar >> 7] * P + (node_ar & 127)

    src_n = node_new[src]
    dst_n = node_new[dst]

    blk = dst_n >> 7                      # new global block
    m_arr = blk // nb
    b_arr = blk % nb                      # position within core
    chunk_id = src_n // CH
    # sorted by (core, block, chunk, src): key-contiguous for `within`
    perm = np.lexsort((src_n, chunk_id, blk))
    # re-derive per-edge fields in sorted order
    src_s = src_n[perm]
    m_s = m_arr[perm]
    b_s = b_arr[perm]
    g_s = b_s // G
    c_s = chunk_id[perm]
    loc_s = (dst_n[perm] & 127).astype(np.float32)

    key = (m_s * nb + b_s) * nch + c_s
    counts = np.bincount(key, minlength=n_blocks * nch).reshape(
        ncores, nb, nch)
    cap = (-(-counts // P)).max(axis=0)   # [nb, nch] cross-core capacity
    geom = Geometry(cfg, cap)

    # slot of each edge within its group
    sub_base_arr = np.zeros((nb, nch), np.int64)
    for g in range(cfg.n_groups):
        for c in range(nch):
            for i, b in enumerate(geom.grp_blocks[g]):
                sub_base_arr[b, c] = geom.sub_base[g][c][i]
    ecum = np.zeros(n_blocks * nch + 1, np.int64)
    np.cumsum(counts.reshape(-1), out=ecum[1:])
    within = np.arange(len(src_s)) - ecum[key]
    slot = sub_base_arr[b_s, c_s] * P + within

    Rmax = geom.Rmax
    loc = np.full((ncores, cfg.n_groups, P, Rmax), 1000.0, np_bf16)
    t_arr = slot // P
    p_arr = slot % P
    loc[m_s, g_s, p_arr, t_arr] = loc_s.astype(np_bf16)

    val = (src_s - c_s * CH).astype(np.int16)
    c16 = slot // 16
    r16 = slot % 16
    flat = np.zeros((ncores, cfg.n_groups, 16, Rmax * 8), np.int16)
    flat[m_s, g_s, r16, c16] = val
    idx16 = np.tile(flat, (1, 1, 8, 1))
    return geom, node_new, idx16, loc


def build_program(cfg: Cfg, geom: Geometry, single_core_sim=False):
    F = cfg.in_feats
    NB, NPC, NPAD = cfg.nb, cfg.npc, cfg.npad
    NCH, CH, NG = cfg.n_chunks, cfg.chunk, cfg.n_groups
    NCLS = cfg.num_classes
    Rmax, capmax = geom.Rmax, geom.capmax

    n_dev = 1 if single_core_sim else cfg.n_cores
    nc = bacc.Bacc("TRN2", target_bir_lowering=False, debug=False,
                   num_devices=n_dev)

    xT = nc.declare_dram_parameter("xT", [F, NPC], f32, isOutput=False)
    W1 = nc.declare_dram_parameter("W1", [F, F], f32, isOutput=False)
    W2 = nc.declare_dram_parameter("W2", [F, F], f32, isOutput=False)
    Wc = nc.declare_dram_parameter("Wc", [F, NCLS], f32, isOutput=False)
    b1 = nc.declare_dram_parameter("b1", [F, 1], f32, isOutput=False)
    b2 = nc.declare_dram_parameter("b2", [F, 1], f32, isOutput=False)
    bc = nc.declare_dram_parameter("bc", [P, NCLS], f32, isOutput=False)
    # per-node row scales, laid out [P, NB] (column c = node block c)
    souts = nc.declare_dram_parameter("souts", [P, NB], f32, isOutput=False)
    sins = nc.declare_dram_parameter("sins", [P, NB], f32, isOutput=False)
    idx16 = nc.declare_dram_parameter("idx16", [NG, P, Rmax * 8], i16,
                                      isOutput=False)
    locm = nc.declare_dram_parameter("locm", [NG, P, Rmax], bf16,
                                     isOutput=False)
    iota_w = nc.declare_dram_parameter("iota_w", [P, capmax * P], bf16,
                                       isOutput=False)
    ident = nc.declare_dram_parameter("ident", [P, P], f32, isOutput=False)
    logits = nc.declare_dram_parameter("logits", [NPC, NCLS], f32,
                                       isOutput=True)

    with tile.TileContext(nc) as tc:
        with (
            tc.tile_pool(name="dram", bufs=1, space="DRAM") as dram,
            tc.tile_pool(name="consts", bufs=1) as consts,
            tc.tile_pool(name="hT", bufs=1) as hTp,
            tc.tile_pool(name="lhs", bufs=3) as lhsp,
            tc.tile_pool(name="gst", bufs=3) as gstp,
            tc.tile_pool(name="meta", bufs=3) as metap,
            tc.tile_pool(name="gat", bufs=3) as gatp,
            tc.tile_pool(name="oh", bufs=4) as ohp,
            tc.tile_pool(name="t1", bufs=3) as t1p,
            tc.tile_pool(name="hsl", bufs=3) as hslp,
            tc.tile_pool(name="out", bufs=3) as outp,
            tc.tile_pool(name="psA", bufs=2, space="PSUM") as psA,
            tc.tile_pool(name="psB", bufs=3, space="PSUM") as psB,
            tc.tile_pool(name="psT", bufs=2, space="PSUM") as psT,
            tc.tile_pool(name="psC", bufs=1, space="PSUM") as psC,
        ):
            # message tables: bf16 pair (hi | lo) per node row, 512B rows
            g_loc = dram.tile([NPC, 2 * F], bf16, name="g_loc")
            g1_full = dram.tile([NPAD, 2 * F], bf16, addr_space="Shared",
                                name="g1_full")
            g2_full = dram.tile([NPAD, 2 * F], bf16, addr_space="Shared",
                                name="g2_full")

            W1_sb = consts.tile([F, F], f32, name="W1_sb")
            nc.sync.dma_start(W1_sb[:], W1[:])
            W2_sb = consts.tile([F, F], f32, name="W2_sb")
            nc.sync.dma_start(W2_sb[:], W2[:])
            Wc_sb = consts.tile([F, NCLS], f32, name="Wc_sb")
            nc.sync.dma_start(Wc_sb[:], Wc[:])
            b1_sb = consts.tile([F, 1], f32, name="b1_sb")
            nc.sync.dma_start(b1_sb[:], b1[:])
            b2_sb = consts.tile([F, 1], f32, name="b2_sb")
            nc.sync.dma_start(b2_sb[:], b2[:])
            bc_sb = consts.tile([P, NCLS], f32, name="bc_sb")
            nc.sync.dma_start(bc_sb[:], bc[:])
            iota_sb = consts.tile([P, capmax * P], bf16, name="iota_sb")
            nc.sync.dma_start(iota_sb[:], iota_w[:])
            ident_sb = consts.tile([P, P], f32, name="ident_sb")
            nc.sync.dma_start(ident_sb[:], ident[:])
            souts_sb = consts.tile([P, NB], f32, name="souts_sb")
            nc.sync.dma_start(souts_sb[:], souts[:])
            sins_sb = consts.tile([P, NB], f32, name="sins_sb")
            nc.sync.dma_start(sins_sb[:], sins[:])

            # h1T split into per-group tiles so layer-2 stage A chunks only
            # depend on their own group's stage-B1 output
            GQ = cfg.group
            h1T = [hTp.tile([F, len(geom.grp_blocks[g]) * P], f32,
                            name=f"h1T_{g}", tag=f"hT{g}")
                   for g in range(NG)]

            def h1T_slice(c):
                g, i = c // GQ, c % GQ
                return h1T[g][:, i * P:(i + 1) * P]

            def stage_a(layer, W_sb, g_dst, chunks=None):
                for c in (range(NB) if chunks is None else chunks):
                    if layer == 1:
                        lhsT = lhsp.tile([F, P], f32, name="lhsT", tag="lhsT")
                        nc.scalar.dma_start(lhsT[:], xT[:, c * P:(c + 1) * P])
                        lhs_ap = lhsT[:]
                    else:
                        lhs_ap = h1T_slice(c)
                    pa = psA.tile([P, F], f32, name="pa", tag="pa")
                    nc.tensor.matmul(pa[:], lhs_ap, W_sb[:], start=True,
                                     stop=True)
                    gst = gstp.tile([P, F], f32, name="gst", tag="gst")
                    nc.scalar.activation(
                        out=gst[:], in_=pa[:],
                        func=mybir.ActivationFunctionType.Copy,
                        scale=souts_sb[:, c:c + 1])
                    # split into bf16 (hi | lo) pair: hi = bf16(g),
                    # lo = bf16(g - hi) -> hi + lo ~ g to ~2^-16 rel
                    g2t = gstp.tile([P, 2 * F], bf16, name="g2t", tag="g2t")
                    nc.vector.tensor_copy(out=g2t[:, :F], in_=gst[:])
                    nc.vector.tensor_tensor(
                        out=g2t[:, F:], in0=gst[:], in1=g2t[:, :F],
                        op=mybir.AluOpType.subtract)
                    nc.scalar.dma_start(g_dst[c * P:(c + 1) * P, :], g2t[:])

            def stage_b(layer, g_full, b_sb, hT_out, post_group=None):
                for g in range(NG):
                    Rg = geom.Rg[g]
                    idx = metap.tile([P, Rmax * 8], i16, name="idx",
                                     tag="idx")
                    nc.sync.dma_start(idx[:, :Rg * 8], idx16[g, :, :Rg * 8])
                    loc = metap.tile([P, Rmax], bf16, name="loc", tag="loc")
                    nc.sync.dma_start(loc[:, :Rg], locm[g, :, :Rg])
                    gat = gatp.tile([P, Rmax * 2 * F], bf16, name="gat",
                                    tag="gat")
                    MAXSUB = 64  # calls capped by packing below
                    for c in range(NCH):
                        c0, c1 = geom.chunk_rng[g][c]
                        for s0 in range(c0, c1, MAXSUB):
                            s1 = min(s0 + MAXSUB, c1)
                            n_idx = (s1 - s0) * P
                            out_ap = gat[:, s0 * 2 * F:s1 * 2 * F].rearrange(
                                "p (s f) -> p s f", s=s1 - s0)
                            nc.gpsimd.dma_gather(
                                out_ap=out_ap,
                                in_ap=g_full[c * CH:min((c + 1) * CH, NPAD), :],
                                idxs_ap=idx[:, s0 * 8:s1 * 8],
                                num_idxs=n_idx,
                                num_idxs_reg=n_idx,
                                elem_size=2 * F,
                                single_packet=False,
                            )
                    for i, b in enumerate(geom.grp_blocks[g]):
                        segs = geom.blk_segs[b]
                        pb = psB.tile([P, F], f32, name="pb", tag="pb")
                        n_segs = len(segs)
                        ti = 0
                        n_sub = sum(n for _, n in segs)
                        for s0, n in segs:
                            oh = ohp.tile([P, capmax * P], bf16, name="oh",
                                          tag="oh")
                            nc.vector.tensor_tensor(
                                out=oh[:, :n * P].rearrange(
                                    "p (s f) -> p s f", s=n),
                                in0=iota_sb[:, :n * P].rearrange(
                                    "p (s f) -> p s f", s=n),
                                in1=loc[:, s0:s0 + n].to_broadcast([P, n, P]),
                                op=mybir.AluOpType.is_equal)
                            for k in range(n):
                                t = s0 + k
                                nc.tensor.matmul(
                                    pb[:], oh[:, k * P:(k + 1) * P],
                                    gat[:, t * 2 * F:t * 2 * F + F],
                                    start=(ti == 0), stop=False)
                                nc.tensor.matmul(
                                    pb[:], oh[:, k * P:(k + 1) * P],
                                    gat[:, t * 2 * F + F:(t + 1) * 2 * F],
                                    start=False, stop=(ti == n_sub - 1))
                                ti += 1
                        t1 = t1p.tile([P, F], f32, name="t1", tag="t1")
                        nc.scalar.activation(
                            out=t1[:], in_=pb[:],
                            func=mybir.ActivationFunctionType.Copy,
                            scale=sins_sb[:, b:b + 1])
                        pt = psT.tile([F, P], f32, name="pt", tag="pt")
                        nc.tensor.transpose(pt[:], t1[:], ident_sb[:])
                        if layer == 1:
                            nc.scalar.activation(
                                out=h1T[g][:, i * P:(i + 1) * P], in_=pt[:],
                                func=mybir.ActivationFunctionType.Relu,
                                bias=b_sb[:, :1])
                        else:
                            hsl = hslp.tile([F, P], f32, name="hsl",
                                            tag="hsl")
                            nc.scalar.activation(
                                out=hsl[:], in_=pt[:],
                                func=mybir.ActivationFunctionType.Relu,
                                bias=b_sb[:, :1])
                            pc = psC.tile([P, NCLS], f32, name="pc",
                                          tag="pc")
                            nc.tensor.matmul(pc[:], hsl[:], Wc_sb[:],
                                             start=True, stop=True)
                            o = outp.tile([P, NCLS], f32, name="o", tag="o")
                            nc.vector.tensor_tensor(
                                out=o[:], in0=pc[:], in1=bc_sb[:],
                                op=mybir.AluOpType.add)
                            nc.sync.dma_start(
                                logits[b * P:(b + 1) * P, :], o[:])
                    if post_group is not None:
                        post_group(g)

            def all_gather(g_full):
                if single_core_sim:
                    nc.sync.dma_start(g_full[:NPC, :], g_loc[:])
                else:
                    nc.gpsimd.collective_compute(
                        "AllGather", mybir.AluOpType.bypass,
                        replica_groups=[list(range(cfg.n_cores))],
                        ins=[g_loc[:]], outs=[g_full[:]])

            stage_a(1, W1_sb, g_loc)
            all_gather(g1_full)
            stage_b(1, g1_full, b1_sb, h1T)
            stage_a(2, W2_sb, g_loc)
            all_gather(g2_full)
            stage_b(2, g2_full, b2_sb, None)

    nc.compile()
    return nc


def run(cfg: Cfg, features, src, dst, W1, b1, W2, b2, Wc, bc,
        trace=False, return_results=False):
    F, NPC, NPAD = cfg.in_feats, cfg.npc, cfg.npad
    n = cfg.n_nodes
    src = np.asarray(src).astype(np.int64)
    dst = np.asarray(dst).astype(np.int64)
    features = np.asarray(features, np.float32)
    deg_out = np.bincount(src, minlength=NPAD).astype(np.float32)
    deg_in = np.bincount(dst, minlength=NPAD).astype(np.float32)
    s_out_old = 1.0 / np.sqrt(np.maximum(deg_out, 1.0))
    s_in_old = 1.0 / np.sqrt(np.maximum(deg_in, 1.0))

    geom, node_new, idx16, loc = preprocess(cfg, src, dst)

    x_new = np.zeros((NPAD, F), np.float32)
    x_new[node_new[:n]] = features
    s_out = np.ones(NPAD, np.float32)
    s_out[node_new] = s_out_old
    s_in = np.ones(NPAD, np.float32)
    s_in[node_new] = s_in_old
    xT_full = np.ascontiguousarray(x_new.T)

    iota_np = np.tile(np.arange(P, dtype=np_bf16), (P, geom.capmax))
    ident_np = np.eye(P, dtype=np.float32)
    bc_b = np.tile(np.asarray(bc, np.float32)[None, :], (P, 1))

    in_maps = []
    for m in range(cfg.n_cores):
        sl = slice(m * NPC, (m + 1) * NPC)
        in_maps.append({
            "xT": np.ascontiguousarray(xT_full[:, sl]),
            "W1": np.asarray(W1, np.float32),
            "W2": np.asarray(W2, np.float32),
            "Wc": np.asarray(Wc, np.float32),
            "b1": np.asarray(b1, np.float32)[:, None],
            "b2": np.asarray(b2, np.float32)[:, None],
            "bc": bc_b,
            "souts": np.ascontiguousarray(
                s_out[sl].reshape(cfg.nb, P).T),
            "sins": np.ascontiguousarray(
                s_in[sl].reshape(cfg.nb, P).T),
            "idx16": idx16[m],
            "locm": loc[m],
            "iota_w": iota_np,
            "ident": ident_np,
        })

    nc = build_program(cfg, geom)
    last_err = None
    for _attempt in range(3):
        try:
            res = run_bass_kernel_spmd(nc, in_maps, list(range(cfg.n_cores)),
                                       trace=trace)
            break
        except Exception as e:  # transient axon worker hiccups
            last_err = e
    else:
        raise last_err
    out_new = np.concatenate([r["logits"] for r in res.results], axis=0)
    out = out_new[node_new[:n]].astype(np.float32)
    if return_results:
        return out, res
    return out


def kernel(features, src, dst, W1, b1, W2, b2, Wc, bc):
    return run(CFG, features, src, dst, W1, b1, W2, b2, Wc, bc)



# revision 9
# speedup vs baseline: 1.3261x; 1.3261x over previous
"""Trainium2 Bass kernel for a 2-layer GCN (DGL GraphConv, norm='both').

Reference computation (per layer):
    h = relu( deg_in^-0.5 * segment_sum( ((x * deg_out^-0.5) @ W)[src], dst ) + b )
then logits = h2 @ Wc + bc.

Distribution: nodes are relabeled into 128-wide blocks, blocks are
load-balanced across the 8 NeuronCores (snake assignment by edge count),
giving every core an equal, structurally identical workload (SPMD: one
program, per-core data). Per layer:
  stage A: each core computes g = (x @ W) * s_out for its node shard,
  rounded to bf16 (256B rows)
  AllGather: g shards -> full g table in every core's DRAM
  stage B: blocks are processed in supergroups of SG blocks; per supergroup
    one dma_gather per source chunk pulls all edge messages; the per-block
    segment-sum is one-hot x messages matmuls accumulated in PSUM; epilogue
    scales by s_in, transposes, adds bias, relu -> h^T kept in SBUF.
Layer 2's epilogue is fused with the classifier: logits = h2 @ Wc + bc.

Gather-table chunking: int16 gather indices reach 32768 rows, chunks start
every 25088 rows, so the per-(block,chunk) cell boundary is flexible within
the 7680-row overlap.  Cell sizes for chunks 0..2 are fixed multiples of 128
shared across cores (edges slide between adjacent cells to fill them
exactly); only the last cell pays cross-core max + round-to-128 padding.
Gather metadata (idx16/loc) is identical for both layers and kept resident
in SBUF.

All index preprocessing (degree counts, edge sorting/padding, relabeling)
is host-side numpy on integer graph structure; float math is on device.
"""
import math
from dataclasses import dataclass

import numpy as np

import concourse.bacc as bacc
import concourse.mybir as mybir
import concourse.tile as tile
from concourse.bass_utils import run_bass_kernel_spmd

f32 = mybir.dt.float32
bf16 = mybir.dt.bfloat16
i16 = mybir.dt.int16

P = 128  # partitions / node block size

# numpy view of bfloat16 for host-side constant/input arrays
import ml_dtypes  # noqa: E402  (ships with jax)

np_bf16 = ml_dtypes.bfloat16


@dataclass
class Cfg:
    n_nodes: int = 100000
    in_feats: int = 128
    num_classes: int = 4
    n_cores: int = 8
    nb: int = 98          # node blocks per core
    chunk: int = 25088    # gather chunk stride (table rows)
    window: int = 32768   # int16 gather reach from a chunk base
    sg: int = 7           # blocks per supergroup (one gather round)

    @property
    def npc(self):        # nodes per core
        return self.nb * P

    @property
    def npad(self):       # padded node count
        return self.n_cores * self.npc

    @property
    def n_chunks(self):
        return math.ceil(self.npad / self.chunk)

    @property
    def n_sg(self):
        return math.ceil(self.nb / self.sg)


CFG = Cfg()


class Geometry:
    """Static slot layout shared by all cores.

    caps[b, c] = subtiles of cell (block position b, chunk c).  Supergroup
    layout is chunk-major: for sg, subtile order is (c=0: blocks sg*SG..,
    c=1: blocks.., ...) so each (sg, chunk) is one contiguous gather."""

    def __init__(self, cfg: Cfg, caps: np.ndarray):
        self.caps = caps  # [nb, n_chunks] int
        SG, NSG, NCH = cfg.sg, cfg.n_sg, cfg.n_chunks
        self.sg_blocks = [list(range(s * SG, min((s + 1) * SG, cfg.nb)))
                          for s in range(NSG)]
        self.sub_base = []   # [NSG][NCH][i] subtile base (sg-relative)
        self.chunk_rng = []  # [NSG][NCH] (s0, s1) subtile range
        self.Rsg = []        # [NSG] total subtiles
        for s in range(NSG):
            blocks = self.sg_blocks[s]
            base, rng, t = [], [], 0
            for c in range(NCH):
                row, c0 = [], t
                for b in blocks:
                    row.append(t)
                    t += int(caps[b, c])
                base.append(row)
                rng.append((c0, t))
            self.sub_base.append(base)
            self.chunk_rng.append(rng)
            self.Rsg.append(t)
        self.Rmax = max(self.Rsg)
        self.capmax = int(caps.max())
        # resident metadata column offsets per sg (in subtiles)
        self.sg_off = np.concatenate([[0], np.cumsum(self.Rsg)]).astype(int)
        self.total_sub = int(self.sg_off[-1])


def preprocess(cfg: Cfg, src: np.ndarray, dst: np.ndarray):
    """Relabel nodes, assign edges to (cell, slot), build gather metadata.

    Returns (geom, node_new, idx16, loc):
      idx16: [n_cores, P, total_sub*8] int16 (16-wrapped, 8x replicated)
      loc:   [n_cores, P, total_sub]  bf16 local dst 0..127, 1000 for pads
    Slot j of supergroup s -> partition j%128, subtile j//128.
    """
    ncores, nb, nch = cfg.n_cores, cfg.nb, cfg.n_chunks
    CH, W, SG = cfg.chunk, cfg.window, cfg.sg
    n_blocks = ncores * nb

    # block load balancing: snake-assign blocks by edge count
    blk_tot = np.bincount(dst >> 7, minlength=n_blocks)
    order = np.argsort(-blk_tot, kind="stable")
    rank = np.arange(n_blocks)
    lane = rank % ncores
    rev = (rank // ncores) % 2 == 1
    core_of_rank = np.where(rev, ncores - 1 - lane, lane)
    core_of_old = np.empty(n_blocks, np.int64)
    pos_of_old = np.empty(n_blocks, np.int64)
    core_of_old[order] = core_of_rank
    pos_of_old[order] = rank // ncores
    new_blk_of_old = core_of_old * nb + pos_of_old
    node_ar = np.arange(cfg.npad, dtype=np.int64)
    node_new = new_blk_of_old[node_ar >> 7] * P + (node_ar & 127)

    src_n = node_new[src]
    dst_n = node_new[dst]

    blk = dst_n >> 7
    perm = np.lexsort((src_n, blk))
    src_s = src_n[perm]
    dst_s = dst_n[perm]
    blk_s = blk[perm]
    # per-(core, block) edge ranges in the sorted stream
    cnt_mb = np.bincount(blk_s, minlength=n_blocks)
    off_mb = np.concatenate([[0], np.cumsum(cnt_mb)])

    bases = np.array([c * CH for c in range(nch)])
    # counts below next-chunk base / below window end, per (m, b, c)
    lowc = np.zeros((ncores, nb, nch - 1), np.int64)
    hic = np.zeros((ncores, nb, nch - 1), np.int64)
    for g in range(n_blocks):
        a = src_s[off_mb[g]:off_mb[g + 1]]
        m, b = g // nb, g % nb
        lowc[m, b] = np.searchsorted(a, bases[1:])
        hic[m, b] = np.searchsorted(a, bases[:-1] + W)

    # cumulative cell targets (multiples of 128, shared across cores)
    Ccum = np.zeros((nb, nch - 1), np.int64)
    for c in range(nch - 1):
        need = lowc[:, :, c].max(axis=0)
        Ccum[:, c] = -(-need // P) * P
    for c in range(1, nch - 1):
        Ccum[:, c] = np.maximum(Ccum[:, c], Ccum[:, c - 1])
    # replay the assignment walk arithmetically to size the last cell
    T_mb = cnt_mb.reshape(ncores, nb)
    i0_mb = np.zeros((ncores, nb), np.int64)
    for c in range(nch - 1):
        quota = (Ccum[:, c] - (Ccum[:, c - 1] if c else 0))[None, :]
        take = np.minimum(np.minimum(quota, hic[:, :, c] - i0_mb),
                          T_mb - i0_mb)
        i0_mb += take
    rem = T_mb - i0_mb
    cap_last = (-(-rem // P)).max(axis=0)
    caps = np.zeros((nb, nch), np.int64)
    caps[:, 0] = Ccum[:, 0] // P
    for c in range(1, nch - 1):
        caps[:, c] = (Ccum[:, c] - Ccum[:, c - 1]) // P
    caps[:, nch - 1] = cap_last
    geom = Geometry(cfg, caps)

    total_sub = geom.total_sub
    loc = np.full((ncores, P, total_sub), 1000.0, np_bf16)
    idxflat = np.zeros((ncores, 16, total_sub * 8), np.int16)

    for g in range(n_blocks):
        m, b = g // nb, g % nb
        s = b // SG
        i = b % SG
        a_src = src_s[off_mb[g]:off_mb[g + 1]]
        a_dst = dst_s[off_mb[g]:off_mb[g + 1]]
        T = len(a_src)
        i0 = 0
        for c in range(nch):
            if c < nch - 1:
                quota = int(Ccum[b, c] - (Ccum[b, c - 1] if c else 0))
                avail = int(np.searchsorted(a_src, bases[c] + W)) - i0
                take = min(quota, avail, T - i0)
            else:
                quota = int(caps[b, c]) * P
                take = T - i0
            if quota == 0:
                continue
            # sg-relative slot range for this cell
            sl = np.arange(take) + geom.sub_base[s][c][i] * P
            pp = sl % P
            tt = sl // P + geom.sg_off[s]
            loc[m, pp, tt] = (a_dst[i0:i0 + take] & 127).astype(np_bf16)
            val = (a_src[i0:i0 + take] - bases[c]).astype(np.int16)
            # idx layout: slot j -> row j%16, col subtile*8 + (j%128)//16
            idxflat[m, pp % 16, tt * 8 + pp // 16] = val
            i0 += take
        assert i0 == T, (g, i0, T)

    idx16 = np.tile(idxflat, (1, 8, 1))
    return geom, node_new, idx16, loc


def build_program(cfg: Cfg, geom: Geometry, single_core_sim=False):
    F = cfg.in_feats
    NB, NPC, NPAD = cfg.nb, cfg.npc, cfg.npad
    NCH, CH, W = cfg.n_chunks, cfg.chunk, cfg.window
    NSG, SG = cfg.n_sg, cfg.sg
    NCLS = cfg.num_classes
    TOT = geom.total_sub
    capmax = geom.capmax

    n_dev = 1 if single_core_sim else cfg.n_cores
    nc = bacc.Bacc("TRN2", target_bir_lowering=False, debug=False,
                   num_devices=n_dev)

    xT = nc.declare_dram_parameter("xT", [F, NPC], bf16, isOutput=False)
    W1 = nc.declare_dram_parameter("W1", [F, F], bf16, isOutput=False)
    W2 = nc.declare_dram_parameter("W2", [F, F], bf16, isOutput=False)
    Wc = nc.declare_dram_parameter("Wc", [F, NCLS], bf16, isOutput=False)
    b1 = nc.declare_dram_parameter("b1", [F, 1], f32, isOutput=False)
    b2 = nc.declare_dram_parameter("b2", [F, 1], f32, isOutput=False)
    bc = nc.declare_dram_parameter("bc", [P, NCLS], f32, isOutput=False)
    souts = nc.declare_dram_parameter("souts", [P, NB], f32, isOutput=False)
    sins = nc.declare_dram_parameter("sins", [P, NB], f32, isOutput=False)
    idx16 = nc.declare_dram_parameter("idx16", [P, TOT * 8], i16,
                                      isOutput=False)
    locm = nc.declare_dram_parameter("locm", [P, TOT], bf16, isOutput=False)
    iota_w = nc.declare_dram_parameter("iota_w", [P, capmax * P], bf16,
                                       isOutput=False)
    ident = nc.declare_dram_parameter("ident", [P, P], bf16, isOutput=False)
    logits = nc.declare_dram_parameter("logits", [NPC, NCLS], f32,
                                       isOutput=True)

    with tile.TileContext(nc) as tc:
        with (
            tc.tile_pool(name="dram", bufs=1, space="DRAM") as dram,
            tc.tile_pool(name="consts", bufs=1) as consts,
            tc.tile_pool(name="hT", bufs=1) as hTp,
            tc.tile_pool(name="lhs", bufs=3) as lhsp,
            tc.tile_pool(name="gst", bufs=3) as gstp,
            tc.tile_pool(name="gat", bufs=3) as gatp,
            tc.tile_pool(name="oh", bufs=4) as ohp,
            tc.tile_pool(name="t1", bufs=3) as t1p,
            tc.tile_pool(name="hsl", bufs=3) as hslp,
            tc.tile_pool(name="out", bufs=3) as outp,
            tc.tile_pool(name="psA", bufs=2, space="PSUM") as psA,
            tc.tile_pool(name="psB", bufs=3, space="PSUM") as psB,
            tc.tile_pool(name="psT", bufs=2, space="PSUM") as psT,
            tc.tile_pool(name="psC", bufs=1, space="PSUM") as psC,
        ):
            # message tables: bf16 rows, 256B each
            g_loc = dram.tile([NPC, F], bf16, name="g_loc")
            g1_full = dram.tile([NPAD, F], bf16, addr_space="Shared",
                                name="g1_full")
            g2_full = dram.tile([NPAD, F], bf16, addr_space="Shared",
                                name="g2_full")

            W1_sb = consts.tile([F, F], bf16, name="W1_sb")
            nc.sync.dma_start(W1_sb[:], W1[:])
            W2_sb = consts.tile([F, F], bf16, name="W2_sb")
            nc.sync.dma_start(W2_sb[:], W2[:])
            Wc_sb = consts.tile([F, NCLS], bf16, name="Wc_sb")
            nc.sync.dma_start(Wc_sb[:], Wc[:])
            b1_sb = consts.tile([F, 1], f32, name="b1_sb")
            nc.sync.dma_start(b1_sb[:], b1[:])
            b2_sb = consts.tile([F, 1], f32, name="b2_sb")
            nc.sync.dma_start(b2_sb[:], b2[:])
            bc_sb = consts.tile([P, NCLS], f32, name="bc_sb")
            nc.sync.dma_start(bc_sb[:], bc[:])
            iota_sb = consts.tile([P, capmax * P], bf16, name="iota_sb")
            nc.sync.dma_start(iota_sb[:], iota_w[:])
            ident_sb = consts.tile([P, P], bf16, name="ident_sb")
            nc.sync.dma_start(ident_sb[:], ident[:])
            souts_sb = consts.tile([P, NB], f32, name="souts_sb")
            nc.sync.dma_start(souts_sb[:], souts[:])
            sins_sb = consts.tile([P, NB], f32, name="sins_sb")
            nc.sync.dma_start(sins_sb[:], sins[:])
            # resident gather metadata (shared by both layers)
            idx_sb = consts.tile([P, TOT * 8], i16, name="idx_sb")
            nc.sync.dma_start(idx_sb[:], idx16[:])
            loc_sb = consts.tile([P, TOT], bf16, name="loc_sb")
            nc.sync.dma_start(loc_sb[:], locm[:])

            # h1T split into per-supergroup tiles so layer-2 stage A blocks
            # only depend on their own supergroup's stage-B1 output
            h1T = [hTp.tile([F, len(geom.sg_blocks[s]) * P], bf16,
                            name=f"h1T_{s}", tag=f"hT{s}")
                   for s in range(NSG)]

            def h1T_slice(c):
                s, i = c // SG, c % SG
                return h1T[s][:, i * P:(i + 1) * P]

            def stage_a(layer, W_sb, g_dst):
                for s in range(NSG):
                    blocks = geom.sg_blocks[s]
                    ns = len(blocks)
                    if layer == 1:
                        lhsT = lhsp.tile([F, SG * P], bf16, name="lhsT",
                                         tag="lhsT")
                        nc.scalar.dma_start(
                            lhsT[:, :ns * P],
                            xT[:, blocks[0] * P:(blocks[0] + ns) * P])
                    g2t = gstp.tile([P, SG * F], bf16, name="g2t", tag="g2t")
                    for i, c in enumerate(blocks):
                        lhs_ap = (lhsT[:, i * P:(i + 1) * P] if layer == 1
                                  else h1T_slice(c))
                        pa = psA.tile([P, F], f32, name="pa", tag="pa")
                        nc.tensor.matmul(pa[:], lhs_ap, W_sb[:], start=True,
                                         stop=True)
                        nc.scalar.activation(
                            out=g2t[:, i * F:(i + 1) * F], in_=pa[:],
                            func=mybir.ActivationFunctionType.Copy,
                            scale=souts_sb[:, c:c + 1])
                    dst_ap = g_dst[blocks[0] * P:(blocks[0] + ns) * P, :]
                    dst_ap = dst_ap.rearrange("(i p) f -> p i f", p=P)
                    nc.scalar.dma_start(
                        dst_ap,
                        g2t[:, :ns * F].rearrange("p (i f) -> p i f", i=ns))

            def stage_b(layer, g_full, b_sb):
                for s in range(NSG):
                    Rs = geom.Rsg[s]
                    o_s = int(geom.sg_off[s])
                    gat = gatp.tile([P, geom.Rmax * F], bf16, name="gat",
                                    tag="gat")
                    for c in range(NCH):
                        c0, c1 = geom.chunk_rng[s][c]
                        if c1 == c0:
                            continue
                        n_idx = (c1 - c0) * P
                        out_ap = gat[:, c0 * F:c1 * F].rearrange(
                            "p (s f) -> p s f", s=c1 - c0)
                        nc.gpsimd.dma_gather(
                            out_ap=out_ap,
                            in_ap=g_full[c * CH:min(c * CH + W, NPAD), :],
                            idxs_ap=idx_sb[:, (o_s + c0) * 8:(o_s + c1) * 8],
                            num_idxs=n_idx,
                            num_idxs_reg=n_idx,
                            elem_size=F,
                            single_packet=False,
                        )
                    if layer == 2:
                        osg = outp.tile([P, SG * NCLS], f32, name="osg",
                                        tag="osg")
                    for i, b in enumerate(geom.sg_blocks[s]):
                        pb = psB.tile([P, F], f32, name="pb", tag="pb")
                        n_sub = int(geom.caps[b].sum())
                        ti = 0
                        for c in range(NCH):
                            n = int(geom.caps[b, c])
                            if n == 0:
                                continue
                            s0 = geom.sub_base[s][c][i]
                            oh = ohp.tile([P, capmax * P], bf16, name="oh",
                                          tag="oh")
                            nc.vector.tensor_tensor(
                                out=oh[:, :n * P].rearrange(
                                    "p (s f) -> p s f", s=n),
                                in0=iota_sb[:, :n * P].rearrange(
                                    "p (s f) -> p s f", s=n),
                                in1=loc_sb[:, o_s + s0:o_s + s0 + n]
                                .to_broadcast([P, n, P]),
                                op=mybir.AluOpType.is_equal)
                            for k in range(n):
                                t = s0 + k
                                nc.tensor.matmul(
                                    pb[:], oh[:, k * P:(k + 1) * P],
                                    gat[:, t * F:(t + 1) * F],
                                    start=(ti == 0), stop=(ti == n_sub - 1))
                                ti += 1
                        t1 = t1p.tile([P, F], bf16, name="t1", tag="t1")
                        nc.scalar.activation(
                            out=t1[:], in_=pb[:],
                            func=mybir.ActivationFunctionType.Copy,
                            scale=sins_sb[:, b:b + 1])
                        pt = psT.tile([F, P], f32, name="pt", tag="pt")
                        nc.tensor.transpose(pt[:], t1[:], ident_sb[:])
                        if layer == 1:
                            nc.scalar.activation(
                                out=h1T[s][:, i * P:(i + 1) * P], in_=pt[:],
                                func=mybir.ActivationFunctionType.Relu,
                                bias=b_sb[:, :1])
                        else:
                            hsl = hslp.tile([F, P], bf16, name="hsl",
                                            tag="hsl")
                            nc.scalar.activation(
                                out=hsl[:], in_=pt[:],
                                func=mybir.ActivationFunctionType.Relu,
                                bias=b_sb[:, :1])
                            pc = psC.tile([P, NCLS], f32, name="pc",
                                          tag="pc")
                            nc.tensor.matmul(pc[:], hsl[:], Wc_sb[:],
                                             start=True, stop=True)
                            nc.vector.tensor_tensor(
                                out=osg[:, i * NCLS:(i + 1) * NCLS],
                                in0=pc[:], in1=bc_sb[:],
                                op=mybir.AluOpType.add)
                    if layer == 2:
                        blocks = geom.sg_blocks[s]
                        ns = len(blocks)
                        dst_ap = logits[blocks[0] * P:(blocks[0] + ns) * P, :]
                        dst_ap = dst_ap.rearrange("(i p) c -> p i c", p=P)
                        nc.sync.dma_start(
                            dst_ap,
                            osg[:, :ns * NCLS].rearrange(
                                "p (i c) -> p i c", i=ns))

            def all_gather(g_full):
                if single_core_sim:
                    nc.sync.dma_start(g_full[:NPC, :], g_loc[:])
                else:
                    nc.gpsimd.collective_compute(
                        "AllGather", mybir.AluOpType.bypass,
                        replica_groups=[list(range(cfg.n_cores))],
                        ins=[g_loc[:]], outs=[g_full[:]])

            stage_a(1, W1_sb, g_loc)
            all_gather(g1_full)
            stage_b(1, g1_full, b1_sb)
            stage_a(2, W2_sb, g_loc)
            all_gather(g2_full)
            stage_b(2, g2_full, b2_sb)

    nc.compile()
    return nc


def run(cfg: Cfg, features, src, dst, W1, b1, W2, b2, Wc, bc,
        trace=False, return_results=False):
    F, NPC, NPAD = cfg.in_feats, cfg.npc, cfg.npad
    n = cfg.n_nodes
    src = np.asarray(src).astype(np.int64)
    dst = np.asarray(dst).astype(np.int64)
    features = np.asarray(features, np.float32)
    deg_out = np.bincount(src, minlength=NPAD).astype(np.float32)
    deg_in = np.bincount(dst, minlength=NPAD).astype(np.float32)
    s_out_old = 1.0 / np.sqrt(np.maximum(deg_out, 1.0))
    s_in_old = 1.0 / np.sqrt(np.maximum(deg_in, 1.0))

    geom, node_new, idx16, loc = preprocess(cfg, src, dst)

    x_new = np.zeros((NPAD, F), np.float32)
    x_new[node_new[:n]] = features
    s_out = np.ones(NPAD, np.float32)
    s_out[node_new] = s_out_old
    s_in = np.ones(NPAD, np.float32)
    s_in[node_new] = s_in_old
    xT_full = np.ascontiguousarray(x_new.T)

    iota_np = np.tile(np.arange(P, dtype=np_bf16), (P, geom.capmax))
    ident_np = np.eye(P, dtype=np_bf16)
    bc_b = np.tile(np.asarray(bc, np.float32)[None, :], (P, 1))

    in_maps = []
    for m in range(cfg.n_cores):
        sl = slice(m * NPC, (m + 1) * NPC)
        in_maps.append({
            "xT": np.ascontiguousarray(xT_full[:, sl]).astype(np_bf16),
            "W1": np.asarray(W1, np.float32).astype(np_bf16),
            "W2": np.asarray(W2, np.float32).astype(np_bf16),
            "Wc": np.asarray(Wc, np.float32).astype(np_bf16),
            "b1": np.asarray(b1, np.float32)[:, None],
            "b2": np.asarray(b2, np.float32)[:, None],
            "bc": bc_b,
            "souts": np.ascontiguousarray(
                s_out[sl].reshape(cfg.nb, P).T),
            "sins": np.ascontiguousarray(
                s_in[sl].reshape(cfg.nb, P).T),
            "idx16": idx16[m],
            "locm": loc[m],
            "iota_w": iota_np,
            "ident": ident_np,
        })

    nc = build_program(cfg, geom)
    last_err = None
    for _attempt in range(3):
        try:
            res = run_bass_kernel_spmd(nc, in_maps, list(range(cfg.n_cores)),
                                       trace=trace)
            break
        except Exception as e:  # transient axon worker hiccups
            last_err = e
    else:
        raise last_err
    out_new = np.concatenate([r["logits"] for r in res.results], axis=0)
    out = out_new[node_new[:n]].astype(np.float32)
    if return_results:
        return out, res
    return out


def kernel(features, src, dst, W1, b1, W2, b2, Wc, bc):
    return run(CFG, features, src, dst, W1, b1, W2, b2, Wc, bc)


# revision 26
# speedup vs baseline: 1.3793x; 1.0401x over previous
"""Trainium2 Bass kernel for a 2-layer GCN (DGL GraphConv, norm='both').

Reference computation (per layer):
    h = relu( deg_in^-0.5 * segment_sum( ((x * deg_out^-0.5) @ W)[src], dst ) + b )
then logits = h2 @ Wc + bc.

Distribution: nodes are relabeled into 128-wide blocks, blocks are
load-balanced across the 8 NeuronCores (snake assignment by edge count),
giving every core an equal, structurally identical workload (SPMD: one
program, per-core data). Per layer:
  stage A: each core computes g = (x @ W) * s_out for its node shard,
  rounded to bf16 (256B rows)
  AllGather: g shards -> full g table in every core's DRAM
  stage B: blocks are processed in supergroups of SG blocks; per supergroup
    one dma_gather per source chunk pulls all edge messages; the per-block
    segment-sum is one-hot x messages matmuls accumulated in PSUM; epilogue
    scales by s_in, transposes, adds bias, relu -> h^T kept in SBUF.
Layer 2's epilogue is fused with the classifier: logits = h2 @ Wc + bc.

Gather-table chunking: int16 gather indices reach 32768 rows, chunks start
every 25088 rows, so the per-(block,chunk) cell boundary is flexible within
the 7680-row overlap.  Cell sizes for chunks 0..2 are fixed multiples of 128
shared across cores (edges slide between adjacent cells to fill them
exactly); only the last cell pays cross-core max + round-to-128 padding.
Gather metadata (idx16/loc) is identical for both layers and kept resident
in SBUF.

All index preprocessing (degree counts, edge sorting/padding, relabeling)
is host-side numpy on integer graph structure; float math is on device.
"""
import math
from dataclasses import dataclass

import numpy as np

import concourse.bacc as bacc
import concourse.mybir as mybir
import concourse.tile as tile
from concourse.bass_utils import run_bass_kernel_spmd

f32 = mybir.dt.float32
bf16 = mybir.dt.bfloat16
i16 = mybir.dt.int16

P = 128  # partitions / node block size

# numpy view of bfloat16 for host-side constant/input arrays
import ml_dtypes  # noqa: E402  (ships with jax)

np_bf16 = ml_dtypes.bfloat16


@dataclass
class Cfg:
    n_nodes: int = 100000
    in_feats: int = 128
    num_classes: int = 4
    n_cores: int = 8
    nb: int = 98          # node blocks per core
    chunk: int = 25088    # gather chunk stride (table rows)
    window: int = 32768   # int16 gather reach from a chunk base
    sg: int = 7           # blocks per supergroup (one gather round)

    @property
    def npc(self):        # nodes per core
        return self.nb * P

    @property
    def npad(self):       # padded node count
        return self.n_cores * self.npc

    @property
    def n_chunks(self):
        return math.ceil(self.npad / self.chunk)

    @property
    def sg_sizes(self):
        full = (self.nb - 7) // self.sg
        rest = self.nb - full * self.sg
        sizes = [self.sg] * full
        while rest > 4:
            sizes.append(4)
            rest -= 4
        if rest:
            sizes.append(rest)
        return sizes

    @property
    def n_sg(self):
        return len(self.sg_sizes)


CFG = Cfg()


class Geometry:
    """Static slot layout shared by all cores.

    caps[b, c] = subtiles of cell (block position b, chunk c).  Supergroup
    layout is chunk-major: for sg, subtile order is (c=0: blocks sg*SG..,
    c=1: blocks.., ...) so each (sg, chunk) is one contiguous gather."""

    def __init__(self, cfg: Cfg, caps: np.ndarray):
        self.caps = caps  # [nb, n_chunks] int
        NSG, NCH = cfg.n_sg, cfg.n_chunks
        self.sg_blocks = []
        b0 = 0
        for sz in cfg.sg_sizes:
            self.sg_blocks.append(list(range(b0, b0 + sz)))
            b0 += sz
        assert b0 == cfg.nb
        self.sub_base = []   # [NSG][NCH][i] subtile base (sg-relative)
        self.chunk_rng = []  # [NSG][NCH] (s0, s1) subtile range
        self.Rsg = []        # [NSG] total subtiles
        for s in range(NSG):
            blocks = self.sg_blocks[s]
            base, rng, t = [], [], 0
            for c in range(NCH):
                row, c0 = [], t
                for b in blocks:
                    row.append(t)
                    t += int(caps[b, c])
                base.append(row)
                rng.append((c0, t))
            self.sub_base.append(base)
            self.chunk_rng.append(rng)
            self.Rsg.append(t)
        self.Rmax = max(self.Rsg)
        self.capmax = int(caps.max())
        # resident metadata column offsets per sg (in subtiles)
        self.sg_off = np.concatenate([[0], np.cumsum(self.Rsg)]).astype(int)
        self.total_sub = int(self.sg_off[-1])
        # block-major loc layout: per block, all its subtiles contiguous
        # (chunk-major within the block) -> one is_equal per block
        nb = cfg.nb
        self.nsub_b = caps.sum(axis=1).astype(int)       # subtiles per block
        self.loc_off = np.concatenate(
            [[0], np.cumsum([self.nsub_b[b] for s in range(NSG)
                             for b in self.sg_blocks[s]])]).astype(int)
        # loc_off is ordered by (sg, block-in-sg) = plain block order
        self.maxsub = int(self.nsub_b.max())
        # per block: gat subtile indices (sg-relative), block-major order
        self.blk_subs = []
        for b in range(nb):
            s = next(si for si, blks in enumerate(self.sg_blocks)
                     if b in blks)
            i = self.sg_blocks[s].index(b)
            subs = []
            for c in range(NCH):
                t0 = self.sub_base[s][c][i]
                subs.extend(range(t0, t0 + int(caps[b, c])))
            self.blk_subs.append(subs)


def preprocess(cfg: Cfg, src: np.ndarray, dst: np.ndarray):
    """Relabel nodes, assign edges to (cell, slot), build gather metadata.

    Returns (geom, node_new, idx16, loc):
      idx16: [n_cores, P, total_sub*8] int16 (16-wrapped, 8x replicated)
      loc:   [n_cores, P, total_sub]  bf16 local dst 0..127, 1000 for pads
    Slot j of supergroup s -> partition j%128, subtile j//128.
    """
    ncores, nb, nch = cfg.n_cores, cfg.nb, cfg.n_chunks
    CH, W = cfg.chunk, cfg.window
    sg_of_b = np.zeros(nb, np.int64)
    i_of_b = np.zeros(nb, np.int64)
    b0 = 0
    for si, sz in enumerate(cfg.sg_sizes):
        sg_of_b[b0:b0 + sz] = si
        i_of_b[b0:b0 + sz] = np.arange(sz)
        b0 += sz
    n_blocks = ncores * nb

    # block load balancing: snake-assign blocks by edge count
    blk_tot = np.bincount(dst >> 7, minlength=n_blocks)
    order = np.argsort(-blk_tot, kind="stable")
    rank = np.arange(n_blocks)
    lane = rank % ncores
    rev = (rank // ncores) % 2 == 1
    core_of_rank = np.where(rev, ncores - 1 - lane, lane)
    core_of_old = np.empty(n_blocks, np.int64)
    pos_of_old = np.empty(n_blocks, np.int64)
    core_of_old[order] = core_of_rank
    pos_of_old[order] = rank // ncores
    new_blk_of_old = core_of_old * nb + pos_of_old
    node_ar = np.arange(cfg.npad, dtype=np.int64)
    node_new = new_blk_of_old[node_ar >> 7] * P + (node_ar & 127)

    src_n = node_new[src]
    dst_n = node_new[dst]

    blk = dst_n >> 7
    perm = np.lexsort((src_n, blk))
    src_s = src_n[perm]
    dst_s = dst_n[perm]
    blk_s = blk[perm]
    # per-(core, block) edge ranges in the sorted stream
    cnt_mb = np.bincount(blk_s, minlength=n_blocks)
    off_mb = np.concatenate([[0], np.cumsum(cnt_mb)])

    bases = np.array([c * CH for c in range(nch)])
    # counts below next-chunk base / below window end, per (m, b, c)
    lowc = np.zeros((ncores, nb, nch - 1), np.int64)
    hic = np.zeros((ncores, nb, nch - 1), np.int64)
    for g in range(n_blocks):
        a = src_s[off_mb[g]:off_mb[g + 1]]
        m, b = g // nb, g % nb
        lowc[m, b] = np.searchsorted(a, bases[1:])
        hic[m, b] = np.searchsorted(a, bases[:-1] + W)

    # cumulative cell targets (multiples of 128, shared across cores)
    Ccum = np.zeros((nb, nch - 1), np.int64)
    for c in range(nch - 1):
        need = lowc[:, :, c].max(axis=0)
        Ccum[:, c] = -(-need // P) * P
    for c in range(1, nch - 1):
        Ccum[:, c] = np.maximum(Ccum[:, c], Ccum[:, c - 1])
    # replay the assignment walk arithmetically to size the last cell
    T_mb = cnt_mb.reshape(ncores, nb)
    i0_mb = np.zeros((ncores, nb), np.int64)
    for c in range(nch - 1):
        quota = (Ccum[:, c] - (Ccum[:, c - 1] if c else 0))[None, :]
        take = np.minimum(np.minimum(quota, hic[:, :, c] - i0_mb),
                          T_mb - i0_mb)
        i0_mb += take
    rem = T_mb - i0_mb
    cap_last = (-(-rem // P)).max(axis=0)
    caps = np.zeros((nb, nch), np.int64)
    caps[:, 0] = Ccum[:, 0] // P
    for c in range(1, nch - 1):
        caps[:, c] = (Ccum[:, c] - Ccum[:, c - 1]) // P
    caps[:, nch - 1] = cap_last
    geom = Geometry(cfg, caps)

    total_sub = geom.total_sub
    loc = np.full((ncores, P, total_sub), 1000.0, np_bf16)
    idxflat = np.zeros((ncores, 16, total_sub * 8), np.int16)

    for g in range(n_blocks):
        m, b = g // nb, g % nb
        s = int(sg_of_b[b])
        i = int(i_of_b[b])
        a_src = src_s[off_mb[g]:off_mb[g + 1]]
        a_dst = dst_s[off_mb[g]:off_mb[g + 1]]
        T = len(a_src)
        i0 = 0
        for c in range(nch):
            if c < nch - 1:
                quota = int(Ccum[b, c] - (Ccum[b, c - 1] if c else 0))
                avail = int(np.searchsorted(a_src, bases[c] + W)) - i0
                take = min(quota, avail, T - i0)
            else:
                quota = int(caps[b, c]) * P
                take = T - i0
            if quota == 0:
                continue
            # sg-relative slot range for this cell
            sl = np.arange(take) + geom.sub_base[s][c][i] * P
            pp = sl % P
            tt = sl // P + geom.sg_off[s]
            # loc is block-major: block's subtiles contiguous
            tloc = (geom.loc_off[b] + int(geom.caps[b, :c].sum())
                    + (sl // P - geom.sub_base[s][c][i]))
            loc[m, pp, tloc] = (a_dst[i0:i0 + take] & 127).astype(np_bf16)
            val = (a_src[i0:i0 + take] - bases[c]).astype(np.int16)
            # idx layout: slot j -> row j%16, col subtile*8 + (j%128)//16
            idxflat[m, pp % 16, tt * 8 + pp // 16] = val
            i0 += take
        assert i0 == T, (g, i0, T)

    idx16 = np.tile(idxflat, (1, 8, 1))
    return geom, node_new, idx16, loc


def build_program(cfg: Cfg, geom: Geometry, single_core_sim=False):
    F = cfg.in_feats
    NB, NPC, NPAD = cfg.nb, cfg.npc, cfg.npad
    NCH, CH, W = cfg.n_chunks, cfg.chunk, cfg.window
    NSG, SG = cfg.n_sg, cfg.sg
    NCLS = cfg.num_classes
    TOT = geom.total_sub
    capmax = geom.capmax

    n_dev = 1 if single_core_sim else cfg.n_cores
    nc = bacc.Bacc("TRN2", target_bir_lowering=False, debug=False,
                   num_devices=n_dev)

    xT = nc.declare_dram_parameter("xT", [F, NPC], bf16, isOutput=False)
    W1 = nc.declare_dram_parameter("W1", [F, F], bf16, isOutput=False)
    W2 = nc.declare_dram_parameter("W2", [F, F], bf16, isOutput=False)
    Wc = nc.declare_dram_parameter("Wc", [F, NCLS], bf16, isOutput=False)
    b1 = nc.declare_dram_parameter("b1", [F, 1], f32, isOutput=False)
    b2 = nc.declare_dram_parameter("b2", [F, 1], f32, isOutput=False)
    bc = nc.declare_dram_parameter("bc", [P, NCLS], f32, isOutput=False)
    souts = nc.declare_dram_parameter("souts", [P, NB], f32, isOutput=False)
    sins = nc.declare_dram_parameter("sins", [P, NB], f32, isOutput=False)
    idx16 = nc.declare_dram_parameter("idx16", [P, TOT * 8], i16,
                                      isOutput=False)
    locm = nc.declare_dram_parameter("locm", [P, TOT], bf16, isOutput=False)
    iota_w = nc.declare_dram_parameter("iota_w", [P, geom.maxsub * P], bf16,
                                       isOutput=False)
    ident = nc.declare_dram_parameter("ident", [P, P], bf16, isOutput=False)
    logits = nc.declare_dram_parameter("logits", [NPC, NCLS], f32,
                                       isOutput=True)

    with tile.TileContext(nc) as tc:
        with (
            tc.tile_pool(name="dram", bufs=1, space="DRAM") as dram,
            tc.tile_pool(name="consts", bufs=1) as consts,
            tc.tile_pool(name="hT", bufs=1) as hTp,
            tc.tile_pool(name="lhs", bufs=3) as lhsp,
            tc.tile_pool(name="gst", bufs=3) as gstp,
            tc.tile_pool(name="gat", bufs=3) as gatp,
            tc.tile_pool(name="oh", bufs=4) as ohp,
            tc.tile_pool(name="t1", bufs=3) as t1p,
            tc.tile_pool(name="hsl", bufs=3) as hslp,
            tc.tile_pool(name="out", bufs=3) as outp,
            tc.tile_pool(name="psA", bufs=2, space="PSUM") as psA,
            tc.tile_pool(name="psB", bufs=3, space="PSUM") as psB,
            tc.tile_pool(name="psT", bufs=2, space="PSUM") as psT,
            tc.tile_pool(name="psC", bufs=1, space="PSUM") as psC,
        ):
            # message tables: bf16 rows, 256B each
            g_loc = dram.tile([NPC, F], bf16, name="g_loc")
            g1_full = dram.tile([NPAD, F], bf16, addr_space="Shared",
                                name="g1_full")
            g2_full = dram.tile([NPAD, F], bf16, addr_space="Shared",
                                name="g2_full")

            W1_sb = consts.tile([F, F], bf16, name="W1_sb")
            nc.sync.dma_start(W1_sb[:], W1[:])
            W2_sb = consts.tile([F, F], bf16, name="W2_sb")
            nc.sync.dma_start(W2_sb[:], W2[:])
            Wc_sb = consts.tile([F, NCLS], bf16, name="Wc_sb")
            nc.sync.dma_start(Wc_sb[:], Wc[:])
            b1_sb = consts.tile([F, 1], f32, name="b1_sb")
            nc.sync.dma_start(b1_sb[:], b1[:])
            b2_sb = consts.tile([F, 1], f32, name="b2_sb")
            nc.sync.dma_start(b2_sb[:], b2[:])
            bc_sb = consts.tile([P, NCLS], f32, name="bc_sb")
            nc.sync.dma_start(bc_sb[:], bc[:])
            iota_sb = consts.tile([P, geom.maxsub * P], bf16, name="iota_sb")
            nc.sync.dma_start(iota_sb[:], iota_w[:])
            ident_sb = consts.tile([P, P], bf16, name="ident_sb")
            nc.sync.dma_start(ident_sb[:], ident[:])
            souts_sb = consts.tile([P, NB], f32, name="souts_sb")
            nc.sync.dma_start(souts_sb[:], souts[:])
            sins_sb = consts.tile([P, NB], f32, name="sins_sb")
            nc.sync.dma_start(sins_sb[:], sins[:])
            # resident gather metadata (shared by both layers)
            idx_sb = consts.tile([P, TOT * 8], i16, name="idx_sb")
            nc.sync.dma_start(idx_sb[:], idx16[:])
            loc_sb = consts.tile([P, TOT], bf16, name="loc_sb")
            nc.sync.dma_start(loc_sb[:], locm[:])

            # h1T split into per-supergroup tiles so layer-2 stage A blocks
            # only depend on their own supergroup's stage-B1 output
            h1T = [hTp.tile([F, len(geom.sg_blocks[s]) * P], bf16,
                            name=f"h1T_{s}", tag=f"hT{s}")
                   for s in range(NSG)]

            sg_of_b = {}
            for s_, blks in enumerate(geom.sg_blocks):
                for i_, b_ in enumerate(blks):
                    sg_of_b[b_] = (s_, i_)

            def h1T_slice(c):
                s, i = sg_of_b[c]
                return h1T[s][:, i * P:(i + 1) * P]

            def stage_a_sg(layer, W_sb, g_dst, s):
                blocks = geom.sg_blocks[s]
                ns = len(blocks)
                g2t = gstp.tile([P, SG * F], bf16, name="g2t", tag="g2t")
                if layer == 1:
                    # s_out is folded into xT host-side: plain batched copies
                    lhsT = lhsp.tile([F, SG * P], bf16, name="lhsT",
                                     tag="lhsT")
                    nc.scalar.dma_start(
                        lhsT[:, :ns * P],
                        xT[:, blocks[0] * P:(blocks[0] + ns) * P])
                    for j0 in range(0, ns, 4):
                        jn = min(4, ns - j0)
                        paw = psA.tile([P, 4 * F], f32, name="paw", tag="pa")
                        for j in range(jn):
                            nc.tensor.matmul(
                                paw[:, j * F:(j + 1) * F],
                                lhsT[:, (j0 + j) * P:(j0 + j + 1) * P],
                                W_sb[:], start=True, stop=True)
                        nc.scalar.activation(
                            out=g2t[:, j0 * F:(j0 + jn) * F],
                            in_=paw[:, :jn * F],
                            func=mybir.ActivationFunctionType.Copy)
                else:
                    for i, c in enumerate(blocks):
                        pa = psA.tile([P, 4 * F], f32, name="pa", tag="pa")
                        nc.tensor.matmul(pa[:, :F], h1T_slice(c), W_sb[:],
                                         start=True, stop=True)
                        nc.scalar.activation(
                            out=g2t[:, i * F:(i + 1) * F], in_=pa[:, :F],
                            func=mybir.ActivationFunctionType.Copy,
                            scale=souts_sb[:, c:c + 1])
                dst_ap = g_dst[blocks[0] * P:(blocks[0] + ns) * P, :]
                dst_ap = dst_ap.rearrange("(i p) f -> p i f", p=P)
                nc.scalar.dma_start(
                    dst_ap,
                    g2t[:, :ns * F].rearrange("p (i f) -> p i f", i=ns))

            def stage_b(layer, g_full, b_sb, interleave=None):
                PRE = 2  # gather prefetch depth (gat pool bufs = PRE + 1)
                gats = {}

                def emit_gather(s):
                    o_s = int(geom.sg_off[s])
                    gat = gatp.tile([P, geom.Rmax * F], bf16, name="gat",
                                    tag="gat")
                    gats[s] = gat
                    for c in range(NCH):
                        c0, c1 = geom.chunk_rng[s][c]
                        if c1 == c0:
                            continue
                        n_idx = (c1 - c0) * P
                        out_ap = gat[:, c0 * F:c1 * F].rearrange(
                            "p (s f) -> p s f", s=c1 - c0)
                        nc.gpsimd.dma_gather(
                            out_ap=out_ap,
                            in_ap=g_full[c * CH:min(c * CH + W, NPAD), :],
                            idxs_ap=idx_sb[:, (o_s + c0) * 8:(o_s + c1) * 8],
                            num_idxs=n_idx,
                            num_idxs_reg=n_idx,
                            elem_size=F,
                            single_packet=False,
                        )

                for s0_ in range(min(PRE, NSG)):
                    emit_gather(s0_)
                for s in range(NSG):
                    if s + PRE < NSG:
                        emit_gather(s + PRE)
                    o_s = int(geom.sg_off[s])
                    gat = gats.pop(s)
                    if layer == 2:
                        osg = outp.tile([P, SG * NCLS], f32, name="osg",
                                        tag="osg")
                    for i, b in enumerate(geom.sg_blocks[s]):
                        pb = psB.tile([P, F], f32, name="pb", tag="pb")
                        n_sub = int(geom.nsub_b[b])
                        l0 = int(geom.loc_off[b])
                        oh = ohp.tile([P, geom.maxsub * P], bf16, name="oh",
                                      tag="oh")
                        nc.vector.tensor_tensor(
                            out=oh[:, :n_sub * P].rearrange(
                                "p (s f) -> p s f", s=n_sub),
                            in0=iota_sb[:, :n_sub * P].rearrange(
                                "p (s f) -> p s f", s=n_sub),
                            in1=loc_sb[:, l0:l0 + n_sub]
                            .to_broadcast([P, n_sub, P]),
                            op=mybir.AluOpType.is_equal)
                        for k, t in enumerate(geom.blk_subs[b]):
                            nc.tensor.matmul(
                                pb[:], oh[:, k * P:(k + 1) * P],
                                gat[:, t * F:(t + 1) * F],
                                start=(k == 0), stop=(k == n_sub - 1))
                        t1 = t1p.tile([P, F], bf16, name="t1", tag="t1")
                        nc.scalar.activation(
                            out=t1[:], in_=pb[:],
                            func=mybir.ActivationFunctionType.Copy,
                            scale=sins_sb[:, b:b + 1])
                        pt = psT.tile([F, P], bf16, name="pt", tag="pt")
                        nc.tensor.transpose(pt[:], t1[:], ident_sb[:])
                        if layer == 1:
                            nc.scalar.activation(
                                out=h1T[s][:, i * P:(i + 1) * P], in_=pt[:],
                                func=mybir.ActivationFunctionType.Relu,
                                bias=b_sb[:, :1])
                        else:
                            hsl = hslp.tile([F, P], bf16, name="hsl",
                                            tag="hsl")
                            nc.scalar.activation(
                                out=hsl[:], in_=pt[:],
                                func=mybir.ActivationFunctionType.Relu,
                                bias=b_sb[:, :1])
                            pc = psC.tile([P, NCLS], f32, name="pc",
                                          tag="pc")
                            nc.tensor.matmul(pc[:], hsl[:], Wc_sb[:],
                                             start=True, stop=True)
                            nc.vector.tensor_tensor(
                                out=osg[:, i * NCLS:(i + 1) * NCLS],
                                in0=pc[:], in1=bc_sb[:],
                                op=mybir.AluOpType.add)
                    if layer == 2:
                        blocks = geom.sg_blocks[s]
                        ns = len(blocks)
                        dst_ap = logits[blocks[0] * P:(blocks[0] + ns) * P, :]
                        dst_ap = dst_ap.rearrange("(i p) c -> p i c", p=P)
                        nc.sync.dma_start(
                            dst_ap,
                            osg[:, :ns * NCLS].rearrange(
                                "p (i c) -> p i c", i=ns))
                    if interleave is not None:
                        interleave(s)

            def all_gather(g_full):
                if single_core_sim:
                    nc.sync.dma_start(g_full[:NPC, :], g_loc[:])
                else:
                    nc.gpsimd.collective_compute(
                        "AllGather", mybir.AluOpType.bypass,
                        replica_groups=[list(range(cfg.n_cores))],
                        ins=[g_loc[:]], outs=[g_full[:]])

            for s in range(NSG):
                stage_a_sg(1, W1_sb, g_loc, s)
            all_gather(g1_full)
            stage_b(1, g1_full, b1_sb,
                    interleave=lambda s: stage_a_sg(2, W2_sb, g_loc, s))
            all_gather(g2_full)
            stage_b(2, g2_full, b2_sb)

    nc.compile()
    return nc


def run(cfg: Cfg, features, src, dst, W1, b1, W2, b2, Wc, bc,
        trace=False, return_results=False):
    F, NPC, NPAD = cfg.in_feats, cfg.npc, cfg.npad
    n = cfg.n_nodes
    src = np.asarray(src).astype(np.int64)
    dst = np.asarray(dst).astype(np.int64)
    features = np.asarray(features, np.float32)
    deg_out = np.bincount(src, minlength=NPAD).astype(np.float32)
    deg_in = np.bincount(dst, minlength=NPAD).astype(np.float32)
    s_out_old = 1.0 / np.sqrt(np.maximum(deg_out, 1.0))
    s_in_old = 1.0 / np.sqrt(np.maximum(deg_in, 1.0))

    geom, node_new, idx16, loc = preprocess(cfg, src, dst)

    x_new = np.zeros((NPAD, F), np.float32)
    x_new[node_new[:n]] = features
    s_out = np.ones(NPAD, np.float32)
    s_out[node_new] = s_out_old
    s_in = np.ones(NPAD, np.float32)
    s_in[node_new] = s_in_old
    xT_full = np.ascontiguousarray((x_new * s_out[:, None]).T)

    iota_np = np.tile(np.arange(P, dtype=np_bf16), (P, geom.maxsub))
    ident_np = np.eye(P, dtype=np_bf16)
    bc_b = np.tile(np.asarray(bc, np.float32)[None, :], (P, 1))

    in_maps = []
    for m in range(cfg.n_cores):
        sl = slice(m * NPC, (m + 1) * NPC)
        in_maps.append({
            "xT": np.ascontiguousarray(xT_full[:, sl]).astype(np_bf16),
            "W1": np.asarray(W1, np.float32).astype(np_bf16),
            "W2": np.asarray(W2, np.float32).astype(np_bf16),
            "Wc": np.asarray(Wc, np.float32).astype(np_bf16),
            "b1": np.asarray(b1, np.float32)[:, None],
            "b2": np.asarray(b2, np.float32)[:, None],
            "bc": bc_b,
            "souts": np.ascontiguousarray(
                s_out[sl].reshape(cfg.nb, P).T),
            "sins": np.ascontiguousarray(
                s_in[sl].reshape(cfg.nb, P).T),
            "idx16": idx16[m],
            "locm": loc[m],
            "iota_w": iota_np,
            "ident": ident_np,
        })

    nc = build_program(cfg, geom)
    last_err = None
    for _attempt in range(3):
        try:
            res = run_bass_kernel_spmd(nc, in_maps, list(range(cfg.n_cores)),
                                       trace=trace)
            break
        except Exception as e:  # transient axon worker hiccups
            last_err = e
    else:
        raise last_err
    out_new = np.concatenate([r["logits"] for r in res.results], axis=0)
    out = out_new[node_new[:n]].astype(np.float32)
    if return_results:
        return out, res
    return out


def kernel(features, src, dst, W1, b1, W2, b2, Wc, bc):
    return run(CFG, features, src, dst, W1, b1, W2, b2, Wc, bc)
